# revision 6
# baseline (speedup 1.0000x reference)
"""DiffSortNet (odd-even transposition soft sort) Trainium2 kernel.

Computes, for vectors [64, 128]:
  x  [64, 128]      softly sorted values
  X  [64, 128, 128] relaxed permutation matrix  (x == vectors @ X)

Sharding: pure data parallel over batch — 8 batches per core on 8 cores.

Per-core algorithm (128 sequential layers):
  x-scan  : pair differences -> ACT arctan (Cauchy CDF) -> alpha -> blend.
            State kept deinterleaved: vE = x[even cols], vO = x[odd cols].
  X-blend : columns of X blended pairwise with the same alpha.  X kept as
            XE/XO [128 rows, 8 batch * 64 cols].  Per layer the alphas
            [8, k] are flattened by DMA to one partition, broadcast to all
            128 row-partitions with a K=1 matmul on the tensor engine, and
            applied with 4 batched tensor_tensor ops on the vector engine.
"""

import sys

sys.path.insert(0, "/opt/trn_rl_repo")

import numpy as np
import concourse.bass as bass
import concourse.mybir as mybir
from concourse.tile import TileContext
from concourse.vector_clock import ScopedClock

F32 = mybir.dt.float32
AF = mybir.ActivationFunctionType
OP = mybir.AluOpType

N = 128          # sorting network width
B = 64           # total batch
NCORES = 8
BL = B // NCORES  # batches per core
K = N // 2        # pairs per even layer
STEEP = 10.0
INV_PI = float(1.0 / np.pi)
# route the two new-X writes to GPSIMD to offload the vector engine
GPS_OFFLOAD = False


class SplitDrainTileContext(TileContext):
    """TileContext that caps semaphore waits per instruction — this walrus
    build rejects instructions carrying more than ~2 sync waits ("Too many
    sync wait commands").  Excess waits are moved onto same-engine NoOp
    instructions inserted immediately before the offender (waits execute in
    program order on the engine's sequencer, so waiting earlier is safe)."""

    MAX_WAITS = 1

    def _drain_and_barrier(self, tick_clock, wait_clock):
        nc = self.nc
        probe = nc.sync.nop(nofuse=True, hint="split_drain_probe")
        wait_clock.add_sem_waits(
            probe.ins, ScopedClock({None: tick_clock.global_clock})
        )
        si = probe.ins.sync_info
        waits = list(si.on_wait) if si and si.on_wait else []
        if len(waits) > 1:
            si.on_wait = waits[:1]
            name2sem = {s.name: s for s in self.sems.allocated().values()}
            for w in waits[1:]:
                nc.sync.wait_ge(name2sem[w.ant_name], w.wait_value)
        nc.sync.drain()
        nc.all_engine_barrier()
        popped = nc._tile_sem_poison_stack.pop()
        assert popped is self._sem_poison
        nc.clear_and_free_semaphores(list(self.sems.allocated().values()))
        nc.all_engine_barrier()
        self._split_excess_waits()

    def _split_excess_waits(self):
        nc = self.nc
        maxw = self.MAX_WAITS
        n = 0
        for f in nc.m.functions:
            for b in f.blocks:
                insts = list(b.instructions)
                out = []
                changed = False
                for inst in insts:
                    si = inst.sync_info
                    waits = list(si.on_wait) if si and si.on_wait else []
                    if len(waits) > maxw:
                        head, keep = waits[:-maxw], waits[-maxw:]
                        for i in range(0, len(head), maxw):
                            nop = mybir.InstNoOp(
                                name=f"I-waitsplit-{n}", ins=[], outs=[]
                            )
                            n += 1
                            nop.engine = inst.engine
                            nop.sync_info = mybir.SyncInfo(
                                on_wait=head[i : i + maxw], on_update=[]
                            )
                            nc.register_instruction(nop, overwrite=True)
                            out.append(nop)
                        si.on_wait = keep
                        changed = True
                    out.append(inst)
                if changed:
                    b.instructions = out


def build_nc():
    nc = bass.Bass("TRN2", target_bir_lowering=False)
    vec = nc.dram_tensor("vectors", [BL, N], F32, kind="ExternalInput")
    xout = nc.dram_tensor("x_out", [BL, N], F32, kind="ExternalOutput")
    Xout = nc.dram_tensor("X_out", [BL, N, N], F32, kind="ExternalOutput")

    with SplitDrainTileContext(nc) as tc:
        with (
            tc.tile_pool(name="const", bufs=1) as cpool,
            tc.tile_pool(name="flow", bufs=2) as fpool,
            tc.tile_pool(name="xflow", bufs=2) as xpool,
            tc.tile_pool(name="psum", bufs=2, space="PSUM") as ppool,
        ):
            # ---- constants / init ----
            ones1 = cpool.tile([1, N], F32, tag="ones1")
            nc.vector.memset(ones1[:, :], 1.0)
            onesf = cpool.tile([N, BL * K], F32, tag="onesf")
            nc.gpsimd.memset(onesf[:, :], 1.0)

            # identity matrix, deinterleaved into even/odd column tiles:
            #   XE[p, b*K + q] = 1 if p == 2q else 0
            #   XO[p, b*K + q] = 1 if p == 2q + 1 else 0
            XE = xpool.tile([N, BL * K], F32, tag="XE")
            XO = xpool.tile([N, BL * K], F32, tag="XO")
            XE3 = XE.rearrange("p (b q) -> p b q", q=K)
            XO3 = XO.rearrange("p (b q) -> p b q", q=K)
            ones3 = onesf.rearrange("p (b q) -> p b q", q=K)
            nc.gpsimd.affine_select(
                XE3[:, :, :], ones3[:, :, :], [[0, BL], [-2, K]],
                OP.is_equal, 0.0, base=0, channel_multiplier=1,
            )
            nc.gpsimd.affine_select(
                XO3[:, :, :], ones3[:, :, :], [[0, BL], [-2, K]],
                OP.is_equal, 0.0, base=-1, channel_multiplier=1,
            )

            # x state, deinterleaved (vE = even cols, vO = odd cols)
            vE = fpool.tile([BL, K], F32, tag="vE")
            vO = fpool.tile([BL, K], F32, tag="vO")
            nc.sync.dma_start(vE[:, :], vec[:, 0:N:2])
            nc.sync.dma_start(vO[:, :], vec[:, 1:N:2])

            # final outputs (interleaved), written by the last layer
            xFull = cpool.tile([BL, N], F32, tag="xFull")
            XFull = cpool.tile([N, BL * N], F32, tag="XFull")
            XFull3 = XFull.rearrange("p (b c) -> p b c", c=N)

            # per-layer X-blend state emitted with one layer of lag so the
            # DVE has X work to do while ACT computes the next arctan
            pend = None  # (l, alpha8, k)

            def emit_xblend(l, alpha8, k):
                last = l == N - 1
                # flatten alphas [BL, k] -> one partition [1, BL*k]
                arow = fpool.tile([1, BL * K], F32, tag="arow")
                a3 = arow.rearrange("p (b q) -> p b q", q=K)
                if k < K:
                    # odd layers: zero the unused q=K-1 slot of each batch
                    # block so the full-width broadcast below reads no
                    # uninitialized memory
                    nc.vector.memset(a3[0:1, :, K - 1 : K], 0.5)
                nc.sync.dma_start(a3[0:1, :, 0:k], alpha8[:, 0:k])
                # broadcast down 128 partitions via K=1 matmul (full width —
                # unused q slots of odd layers carry garbage but are never
                # read by the sliced views below)
                Wp = ppool.tile([N, BL * K], F32, tag="Wp")
                nc.tensor.matmul(
                    Wp[:, :], ones1[:, :], arow[:, :], start=True, stop=True
                )
                Wsb = fpool.tile([N, BL * K], F32, tag="Wsb")
                nc.scalar.copy(Wsb[:, :], Wp[:, :])
                W3 = Wsb.rearrange("p (b q) -> p b q", q=K)

                D = fpool.tile([N, BL * K], F32, tag="D")
                T = fpool.tile([N, BL * K], F32, tag="T")
                nXE = xpool.tile([N, BL * K], F32, tag="XE")
                nXO = xpool.tile([N, BL * K], F32, tag="XO")
                oE3 = XE.rearrange("p (b q) -> p b q", q=K)
                oO3 = XO.rearrange("p (b q) -> p b q", q=K)
                nE3 = nXE.rearrange("p (b q) -> p b q", q=K)
                nO3 = nXO.rearrange("p (b q) -> p b q", q=K)
                if l % 2 == 0:
                    # pairs (2q, 2q+1) = (XE_q, XO_q); A = XE side
                    nc.vector.tensor_tensor(
                        D[:, 0 : BL * K], XO[:, :], XE[:, :], OP.subtract
                    )
                    nc.vector.tensor_tensor(
                        T[:, 0 : BL * K], Wsb[:, 0 : BL * K], D[:, 0 : BL * K],
                        OP.mult,
                    )
                    eng = nc.gpsimd if GPS_OFFLOAD else nc.vector
                    eng.tensor_tensor(
                        nXE[:, :], XO[:, :], T[:, 0 : BL * K], OP.subtract
                    )
                    eng.tensor_tensor(
                        nXO[:, :], XE[:, :], T[:, 0 : BL * K], OP.add
                    )
                else:
                    # pairs (2q+1, 2q+2) = (XO_q, XE_{q+1}), q = 0..62
                    D3 = D.rearrange("p (b q) -> p b q", q=K)
                    T3 = T.rearrange("p (b q) -> p b q", q=K)
                    nc.vector.tensor_tensor(
                        D3[:, :, 0:k], oE3[:, :, 1 : k + 1], oO3[:, :, 0:k],
                        OP.subtract,
                    )
                    nc.vector.tensor_tensor(
                        T3[:, :, 0:k], W3[:, :, 0:k], D3[:, :, 0:k], OP.mult
                    )
                    if not last:
                        eng = nc.gpsimd if GPS_OFFLOAD else nc.vector
                        eng.tensor_tensor(
                            nO3[:, :, 0:k], oE3[:, :, 1 : k + 1], T3[:, :, 0:k],
                            OP.subtract,
                        )
                        eng.tensor_tensor(
                            nE3[:, :, 1 : k + 1], oO3[:, :, 0:k], T3[:, :, 0:k],
                            OP.add,
                        )
                        # untouched boundary columns
                        nc.scalar.copy(nE3[:, :, 0:1], oE3[:, :, 0:1])
                        nc.scalar.copy(nO3[:, :, k : k + 1], oO3[:, :, k : k + 1])
                    else:
                        # write interleaved directly into XFull
                        # new odd cols 2q+1 (q=0..62) and even cols 2q+2
                        nc.vector.tensor_tensor(
                            XFull3[:, :, 1 : 2 * k : 2],
                            oE3[:, :, 1 : k + 1], T3[:, :, 0:k], OP.subtract,
                        )
                        nc.vector.tensor_tensor(
                            XFull3[:, :, 2 : 2 * k + 2 : 2],
                            oO3[:, :, 0:k], T3[:, :, 0:k], OP.add,
                        )
                        nc.scalar.copy(XFull3[:, :, 0:1], oE3[:, :, 0:1])
                        nc.scalar.copy(
                            XFull3[:, :, N - 1 : N], oO3[:, :, k : k + 1]
                        )
                return nXE, nXO

            # ---- the 128 layers ----
            for l in range(N):
                even = l % 2 == 0
                k = K if even else K - 1
                last = l == N - 1
                if even:
                    a_v, b_v = vE[:, 0:K], vO[:, 0:K]
                else:
                    a_v, b_v = vO[:, 0:k], vE[:, 1 : k + 1]

                d = fpool.tile([BL, K], F32, tag="d")
                t = fpool.tile([BL, K], F32, tag="t")
                alpha8 = fpool.tile([BL, K], F32, tag="alpha8")
                u = fpool.tile([BL, K], F32, tag="u")
                nvE = fpool.tile([BL, K], F32, tag="vE")
                nvO = fpool.tile([BL, K], F32, tag="vO")

                nc.vector.tensor_tensor(d[:, 0:k], b_v, a_v, OP.subtract)
                nc.scalar.activation(t[:, 0:k], d[:, 0:k], AF.Arctan, scale=STEEP)
                nc.vector.tensor_scalar(
                    alpha8[:, 0:k], t[:, 0:k], INV_PI, 0.5, OP.mult, OP.add
                )
                nc.vector.tensor_tensor(u[:, 0:k], alpha8[:, 0:k], d[:, 0:k], OP.mult)
                if even:
                    nc.vector.tensor_tensor(nvE[:, :], vO[:, :], u[:, 0:k], OP.subtract)
                    nc.vector.tensor_tensor(nvO[:, :], vE[:, :], u[:, 0:k], OP.add)
                elif not last:
                    nc.vector.tensor_tensor(
                        nvO[:, 0:k], vE[:, 1 : k + 1], u[:, 0:k], OP.subtract
                    )
                    nc.vector.tensor_tensor(
                        nvE[:, 1 : k + 1], vO[:, 0:k], u[:, 0:k], OP.add
                    )
                    nc.scalar.copy(nvE[:, 0:1], vE[:, 0:1])
                    nc.scalar.copy(nvO[:, k : k + 1], vO[:, k : k + 1])
                else:
                    # final layer: write interleaved x directly
                    nc.vector.tensor_tensor(
                        xFull[:, 1 : 2 * k : 2], vE[:, 1 : k + 1], u[:, 0:k],
                        OP.subtract,
                    )
                    nc.vector.tensor_tensor(
                        xFull[:, 2 : 2 * k + 2 : 2], vO[:, 0:k], u[:, 0:k], OP.add
                    )
                    nc.scalar.copy(xFull[:, 0:1], vE[:, 0:1])
                    nc.scalar.copy(xFull[:, N - 1 : N], vO[:, k : k + 1])

                # X-blend of the previous layer (one layer of lag)
                if pend is not None:
                    XE, XO = emit_xblend(*pend)
                pend = (l, alpha8, k)
                if not last:
                    vE, vO = nvE, nvO

            # flush the final layer's X-blend
            emit_xblend(*pend)

            # ---- outputs ----
            nc.sync.dma_start(xout[:, :], xFull[:, :])
            Xv = Xout.rearrange("b r c -> r b c")
            nc.sync.dma_start(Xv[:, :, :], XFull3[:, :, :])

    return nc


def kernel(**inputs):
    from concourse.bass_utils import run_bass_kernel_spmd

    vectors = np.ascontiguousarray(np.asarray(inputs["vectors"], dtype=np.float32))
    assert vectors.shape == (B, N)
    nc = build_nc()
    in_maps = [{"vectors": vectors[c * BL : (c + 1) * BL]} for c in range(NCORES)]
    res = run_bass_kernel_spmd(nc, in_maps, core_ids=list(range(NCORES)))
    x = np.concatenate([res.results[c]["x_out"] for c in range(NCORES)], axis=0)
    X = np.concatenate([res.results[c]["X_out"] for c in range(NCORES)], axis=0)
    return x, X


if __name__ == "__main__":
    rng = np.random.default_rng(0)
    v = rng.standard_normal((B, N), dtype=np.float32)
    x, X = kernel(vectors=v)
    print("x", x.shape, "X", X.shape)
    # quick invariant: x == vectors @ X
    recon = np.einsum("bi,bij->bj", v, X)
    print("invariant max err:", np.abs(recon - x).max())
